# revision 14
# baseline (speedup 1.0000x reference)
"""BasicGCN (2-layer GCN, 100K nodes / 3.2M edges) on 8 Trainium2 NeuronCores.

Strategy (node/dst sharding, graph-parallel), v3:
  - Pad nodes to NPAD = 100352 = 8 * 12544; core c owns dst rows
    [c*12544, (c+1)*12544).
  - Linearity refactor: segment_sum(dinv_s*x_s) @ W1 == the GCN conv, so
    the layer-1 gather table is just xs = dinv*x in bf16 (host-prepared).
    W1/relu/W2 are applied per dst tile in the phase-A epilogue.
  - Self-loops are NOT gather slots: the layer-1 self term (+= xs_own) is
    a per-tile DVE add in the epilogue; the layer-2 self term reuses the
    per-tile y values kept in SBUF (h2keep) from phase A.
  - Edge-granular slot quotas: per (dst-tile, src-group) section quota =
    max over cores of the edge count (no 128 roundup); sections are
    packed back-to-back inside each (tile-quad, group) run, runs padded
    to 128. A 128-slot block that straddles two sections gets TWO one-hot
    matmuls (one per dst tile) using two host-prepared dstloc columns
    (other-tile slots carry a sentinel -> zero S column).
  - Device per core:
      phase A (layer-1 agg): dma_gather xs rows (512B) in <=1024-row
        calls, one-hot S blocks on DVE, segment-sum via PE bf16 matmuls
        into per-tile f32 PSUM; epilogue: agg += xs_own,
        h1 = relu(dinv_d*(agg @ W1) + b1), y = dinv_d*(h1 @ W2) -> bf16
        shard table [12544, 128] (64 data + 64 zero pad) + SBUF copy
        (h2keep).
      AllGather shards -> h2full [NPAD, 128] bf16 (Shared DRAM).
      phase B (layer-2 agg): gather 256B bf16 rows, same S/matmul,
        epilogue adds h2keep self term + log_softmax -> out [12544, 64].
  - Host: concatenate 8 shards, trim to [100000, 64].

Gather tables are split into 4 row-groups of NPAD/4 = 25088 rows so the
int16 gather indices stay in range; each dma_gather call is capped at
QMAX=1024 indices (the q7 firmware breaks above ~1024) and spans a
quad's whole per-group run to keep calls full (SWDGE fixed cost is the
main Pool-engine expense).
"""

import numpy as np

import concourse.bacc as bacc
import concourse.bass as bass
import concourse.mybir as mybir
import concourse.tile as tile
from concourse.bass_utils import run_bass_kernel_spmd

F32 = mybir.dt.float32
BF16 = mybir.dt.bfloat16
I16 = mybir.dt.int16
NP_BF16 = mybir.dt.np(BF16)
AF = mybir.ActivationFunctionType
ALU = mybir.AluOpType

N_CORES = 8
PAD_DSTLOC = 1000.0  # sentinel dst-local for padding slots -> zero S column
QMAX = 1024  # max num_idxs per dma_gather call (HW limit is in (1024, 1280])
QT = 4       # dst tiles per gather bucket (quad)


def make_cfg(n_nodes=100000, d_in=256, d_hid=256, d_out=64, shard_tiles=98,
             n_groups=4):
    shard = shard_tiles * 128
    npad = N_CORES * shard
    assert npad % n_groups == 0
    gr = npad // n_groups
    assert gr <= 32768
    assert n_nodes <= npad
    return dict(N=n_nodes, NPAD=npad, SHARD=shard, NT=shard_tiles,
                NG=n_groups, GR=gr, D_IN=d_in, D_HID=d_hid, D_OUT=d_out)


FULL_CFG = make_cfg()


def _build_schedule(quota, nt, ng):
    """Gather-call schedule over (quad, group) runs, edge-granular quotas.

    Returns dict with:
      calls: list of (g, o, q) in stream order, q <= QMAX, q % 128 == 0
      emissions: per 128-block, list of (tile, dlcol) (1 or 2 entries)
      sec_off: [nt, ng] absolute slot offset of each section
      tile_of_slot: [slot_total] owning tile per slot (-1 for run pad)
      slot_total, n_dlcols
    """
    sec_off = np.zeros((nt, ng), np.int64)
    tile_parts = []
    emissions = []
    calls = []
    n_dlcols = 0
    off = 0
    for qd in range(0, nt, QT):
        tiles = list(range(qd, min(qd + QT, nt)))
        for g in range(ng):
            run_start = off
            for t in tiles:
                q = int(quota[t, g])
                sec_off[t, g] = off
                tile_parts.append(np.full(q, t, np.int32))
                off += q
            pad = (-(off - run_start)) % 128
            if pad:
                tile_parts.append(np.full(pad, -1, np.int32))
                off += pad
            run_len = off - run_start
            if run_len == 0:
                continue
            # blocks + emissions for this run
            run_tiles = np.concatenate(tile_parts[-(len(tiles) + (1 if pad
                                                   else 0)):])
            for b in range(run_len // 128):
                blk_tiles = np.unique(run_tiles[b * 128:(b + 1) * 128])
                blk_tiles = [int(t) for t in blk_tiles if t >= 0]
                assert 1 <= len(blk_tiles) <= 2, blk_tiles
                ent = []
                for t in blk_tiles:
                    ent.append((t, n_dlcols))
                    n_dlcols += 1
                emissions.append(ent)
            # calls chunking the run
            nblk = run_len // 128
            nch = (run_len + QMAX - 1) // QMAX
            base, rem = divmod(nblk, nch)
            o = run_start
            for i in range(nch):
                q = (base + (1 if i < rem else 0)) * 128
                calls.append((g, o, q))
                o += q
    tile_of_slot = np.concatenate(tile_parts)
    assert tile_of_slot.shape[0] == off
    return dict(calls=calls, emissions=emissions, sec_off=sec_off,
                tile_of_slot=tile_of_slot, slot_total=off,
                n_dlcols=n_dlcols)


# --------------------------------------------------------------------------
# Host preprocessing
# --------------------------------------------------------------------------

def preprocess(x, edge_index, W1, b1, W2, b2, cfg):
    N, NPAD, SHARD, NT, NG, GR = (cfg["N"], cfg["NPAD"], cfg["SHARD"],
                                  cfg["NT"], cfg["NG"], cfg["GR"])
    D_IN, D_HID, D_OUT = cfg["D_IN"], cfg["D_HID"], cfg["D_OUT"]

    x = np.asarray(x, np.float32)
    edge_index = np.asarray(edge_index)
    src = edge_index[0].astype(np.int64)
    dst = edge_index[1].astype(np.int64)
    E = src.shape[0]

    # dst-balance permutation: snake-stripe nodes across cores by
    # in-degree so per-(core,tile) edge totals equalize across cores
    indeg = np.bincount(dst, minlength=N)
    iarr = np.arange(N)
    stripe, lane = iarr // N_CORES, iarr % N_CORES
    csel = np.where(stripe % 2 == 0, lane, N_CORES - 1 - lane)
    perm = np.empty(N, np.int64)
    perm[np.argsort(-indeg, kind="stable")] = csel * SHARD + stripe
    src = perm[src]
    dst = perm[dst]

    deg = np.bincount(dst, minlength=NPAD).astype(np.float32) + 1.0
    dinv_pad = 1.0 / np.sqrt(deg)  # [NPAD]; pad rows unused

    c_of = dst // SHARD
    t_of = (dst % SHARD) // 128
    d_of = (dst % 128).astype(np.float32)
    g_of = src // GR
    srcg = (src % GR).astype(np.int16)

    key = (c_of * NT + t_of) * NG + g_of
    order = np.argsort(key, kind="stable")
    counts = np.bincount(key, minlength=N_CORES * NT * NG)
    quota = counts.reshape(N_CORES, NT, NG).max(axis=0)

    sched = _build_schedule(quota, NT, NG)
    slot_total = sched["slot_total"]
    sec_off = sched["sec_off"]

    # slot position of each edge inside its core's stream
    csum = np.zeros(N_CORES * NT * NG + 1, np.int64)
    np.cumsum(counts, out=csum[1:])
    rank = np.arange(E, dtype=np.int64) - csum[key[order]]
    slot = sec_off[t_of[order], g_of[order]] + rank
    core = c_of[order]

    idx_arr = np.zeros((N_CORES, slot_total), np.int16)  # pad -> row 0
    dv_arr = np.full((N_CORES, slot_total), PAD_DSTLOC, np.float32)
    idx_arr[core, slot] = srcg[order]
    dv_arr[core, slot] = d_of[order]

    # idx wrapped [16, slot_total/16] (pos p -> [p%16, p//16]), replicated
    # to 128 partitions
    idx_sb = idx_arr.reshape(N_CORES, slot_total // 16, 16).transpose(
        0, 2, 1)
    idx_sb = np.tile(idx_sb, (1, 8, 1))

    # dstloc columns: one per emission; slots of other tiles -> sentinel
    n_dlcols = sched["n_dlcols"]
    tile_of_slot = sched["tile_of_slot"]
    dl_sb = np.full((N_CORES, 128, n_dlcols), PAD_DSTLOC, np.float32)
    blk = 0
    for ent in sched["emissions"]:
        sl = slice(blk * 128, (blk + 1) * 128)
        tslice = tile_of_slot[sl]
        for t, dlcol in ent:
            dl_sb[:, :, dlcol] = np.where(tslice[None, :] == t,
                                          dv_arr[:, sl], PAD_DSTLOC)
        blk += 1

    # gather table: xs = dinv * x, bf16, row-major (permuted node order)
    xs = np.zeros((NPAD, D_IN), NP_BF16)
    xs[perm] = (dinv_pad[perm][:, None] * x).astype(NP_BF16)

    ntile = NPAD // 128
    dinv_nodes = np.ascontiguousarray(
        dinv_pad.reshape(ntile, 128).T)  # [128, ntile]
    dinv_dst = np.stack([dinv_nodes[:, c * NT:(c + 1) * NT]
                         for c in range(N_CORES)])  # [8, 128, NT]

    iota = np.tile(np.arange(128), (128, 1)).astype(NP_BF16)
    ident_bf = np.eye(128, dtype=NP_BF16)
    b1bc = np.ascontiguousarray(np.broadcast_to(
        np.asarray(b1, np.float32), (128, D_HID))).astype(NP_BF16)
    b2bc = np.ascontiguousarray(
        np.broadcast_to(np.asarray(b2, np.float32), (128, D_OUT)))

    common = dict(xs=xs, W1=np.asarray(W1, NP_BF16),
                  W2=np.asarray(W2, NP_BF16), b1bc=b1bc, b2bc=b2bc,
                  iota=iota, ident_bf=ident_bf)
    in_maps = []
    for c in range(N_CORES):
        m = dict(common)
        m["xs_own"] = np.ascontiguousarray(
            xs[c * SHARD:(c + 1) * SHARD])
        m["dinv_dst"] = np.ascontiguousarray(dinv_dst[c])
        m["idx_sb"] = np.ascontiguousarray(idx_sb[c])
        m["dstloc"] = np.ascontiguousarray(dl_sb[c])
        in_maps.append(m)

    meta = dict(calls=sched["calls"], emissions=sched["emissions"],
                idxcols=slot_total // 16, n_dlcols=n_dlcols, perm=perm)
    return in_maps, meta


# --------------------------------------------------------------------------
# Device program
# --------------------------------------------------------------------------

def build_program(cfg, meta, with_collective=True, phases=(1, 2, 3)):
    NPAD, NT, NG, GR = cfg["NPAD"], cfg["NT"], cfg["NG"], cfg["GR"]
    D_IN, D_HID, D_OUT = cfg["D_IN"], cfg["D_HID"], cfg["D_OUT"]
    SHARD = cfg["SHARD"]
    idxcols, n_dlcols = meta["idxcols"], meta["n_dlcols"]
    calls, emissions = meta["calls"], meta["emissions"]
    KI = D_IN // 128   # k-chunks for W1 matmul
    KH = D_HID // 128  # k-chunks for W2 matmul
    CMAX = QMAX // 128
    D_L2 = 2 * D_OUT  # layer-2 table row: 64 bf16 data + 64 bf16 zeros

    # first/last emission index per tile (accum start/stop + epilogue)
    flat = [(bi, t) for bi, ent in enumerate(emissions) for (t, _) in ent]
    first_em = {}
    last_em = {}
    for i, (_, t) in enumerate(flat):
        first_em.setdefault(t, i)
        last_em[t] = i

    nc = bacc.Bacc("TRN2", target_bir_lowering=False, debug=False,
                   num_devices=N_CORES)

    xs_d = nc.dram_tensor("xs", [NPAD, D_IN], BF16, kind="ExternalInput")
    xso_d = nc.dram_tensor("xs_own", [SHARD, D_IN], BF16,
                           kind="ExternalInput")
    W1_d = nc.dram_tensor("W1", [D_IN, D_HID], BF16, kind="ExternalInput")
    W2_d = nc.dram_tensor("W2", [D_HID, D_OUT], BF16, kind="ExternalInput")
    b1_d = nc.dram_tensor("b1bc", [128, D_HID], BF16, kind="ExternalInput")
    b2_d = nc.dram_tensor("b2bc", [128, D_OUT], F32, kind="ExternalInput")
    iota_d = nc.dram_tensor("iota", [128, 128], BF16, kind="ExternalInput")
    identb_d = nc.dram_tensor("ident_bf", [128, 128], BF16,
                              kind="ExternalInput")
    dinvd_d = nc.dram_tensor("dinv_dst", [128, NT], F32, kind="ExternalInput")
    idx_d = nc.dram_tensor("idx_sb", [128, idxcols], I16, kind="ExternalInput")
    dl_d = nc.dram_tensor("dstloc", [128, n_dlcols], F32,
                          kind="ExternalInput")
    out_d = nc.dram_tensor("out", [SHARD, D_OUT], BF16,
                           kind="ExternalOutput")

    with tile.TileContext(nc) as tc:
        with (
            tc.tile_pool(name="const", bufs=1) as const,
            tc.tile_pool(name="dram", bufs=1, space="DRAM") as dram,
        ):
            h2own = dram.tile([SHARD, D_L2], BF16)
            h2full = dram.tile([NPAD, D_L2], BF16, addr_space="Shared")

            dl_sb = const.tile([128, n_dlcols], F32)
            nc.sync.dma_start(out=dl_sb[:], in_=dl_d.ap())
            idx_sb = const.tile([128, idxcols], I16)
            nc.sync.dma_start(out=idx_sb[:], in_=idx_d.ap())
            w1_sb = const.tile([128, KI, D_HID], BF16)
            for k in range(KI):
                nc.sync.dma_start(out=w1_sb[:, k, :],
                                  in_=W1_d.ap()[k * 128:(k + 1) * 128, :])
            w2_sb = const.tile([128, KH, D_OUT], BF16)
            for k in range(KH):
                nc.sync.dma_start(out=w2_sb[:, k, :],
                                  in_=W2_d.ap()[k * 128:(k + 1) * 128, :])
            iota_sb = const.tile([128, 128], BF16)
            nc.sync.dma_start(out=iota_sb[:], in_=iota_d.ap())
            identb_sb = const.tile([128, 128], BF16)
            nc.sync.dma_start(out=identb_sb[:], in_=identb_d.ap())
            b1_sb = const.tile([128, D_HID], BF16)
            nc.sync.dma_start(out=b1_sb[:], in_=b1_d.ap())
            b2_sb = const.tile([128, D_OUT], F32)
            nc.sync.dma_start(out=b2_sb[:], in_=b2_d.ap())
            dinvd_sb = const.tile([128, NT], F32)
            nc.sync.dma_start(out=dinvd_sb[:], in_=dinvd_d.ap())
            h2keep = const.tile([128, NT, D_OUT], BF16)

            # ---------------- aggregation machinery ----------------------
            h2own_r = h2own.rearrange("(t p) f -> t p f", p=128)
            xso_r = xso_d.ap().rearrange("(t p) f -> t p f", p=128)

            def agg_phase(table, elem, edt, rhs_w, epilogue, mtag, stag,
                          ptag):
                """Gather + one-hot-S + matmul accumulation over the
                precomputed quad-spanning call schedule."""
                blk = 0
                em = 0
                psums = {}
                with (
                    tc.tile_pool(name=mtag, bufs=10) as mpool,
                    tc.tile_pool(name=stag, bufs=8) as spool,
                    tc.tile_pool(name=ptag, bufs=5, space="PSUM") as apsum,
                    tc.tile_pool(name=ptag + "ep", bufs=3) as eppool,
                    tc.tile_pool(name=ptag + "ep2", bufs=1,
                                 space="PSUM") as eppsum,
                ):
                    for g, o, q in calls:
                        ncols = q // 128
                        mt = mpool.tile([128, CMAX, elem], edt, tag="m")
                        nc.gpsimd.dma_gather(
                            mt[:, :ncols, :],
                            table(g),
                            idx_sb[:, o // 16:(o + q) // 16],
                            q, q, elem)
                        for j in range(ncols):
                            for t, dlcol in emissions[blk]:
                                if em == first_em[t]:
                                    psums[t] = apsum.tile(
                                        [128, rhs_w], F32, tag="agg",
                                        name="aggps")
                                st = spool.tile([128, 128], BF16, tag="s",
                                                name="stile")
                                nc.vector.tensor_scalar(
                                    st[:], iota_sb[:],
                                    dl_sb[:, dlcol:dlcol + 1],
                                    None, ALU.is_equal)
                                nc.tensor.matmul(
                                    psums[t][:], st[:], mt[:, j, :rhs_w],
                                    start=(em == first_em[t]),
                                    stop=(em == last_em[t]))
                                if em == last_em[t]:
                                    epilogue(t, psums.pop(t)[:], eppool,
                                             eppsum)
                                em += 1
                            blk += 1

            # -------- phase A: layer-1 agg + full 2-layer epilogue --------
            def epi1(t, ps, eppool, eppsum):
                # ps[d, f] = sum_e dinv_src * x_src  (agg of xs rows)
                xso = eppool.tile([128, D_IN], BF16, tag="xso")
                nc.sync.dma_start(out=xso[:], in_=xso_r[t])
                aggsb = eppool.tile([128, D_IN], BF16, tag="aggsb")
                nc.vector.tensor_tensor(aggsb[:], ps, xso[:], ALU.add)
                tsb = eppool.tile([128, KI, 128], BF16, tag="tsb")
                for k in range(KI):
                    tp = eppsum.tile([128, 128], BF16, tag="tr")
                    nc.tensor.transpose(
                        tp[:], aggsb[:, k * 128:(k + 1) * 128], identb_sb[:])
                    nc.vector.tensor_copy(tsb[:, k, :], tp[:])
                h1ps = eppsum.tile([128, D_HID], F32, tag="h1")
                for k in range(KI):
                    nc.tensor.matmul(h1ps[:], tsb[:, k, :], w1_sb[:, k, :],
                                     start=(k == 0), stop=(k == KI - 1))
                # h1 = relu(dinv_d * (agg @ W1) + b1), bf16
                o1 = eppool.tile([128, D_HID], BF16, tag="o1")
                nc.vector.tensor_scalar(o1[:], h1ps[:], dinvd_sb[:, t:t + 1],
                                        None, ALU.mult)
                nc.vector.tensor_tensor(o1[:], o1[:], b1_sb[:], ALU.add)
                nc.vector.tensor_scalar(o1[:], o1[:], 0.0, None, ALU.max)
                t2sb = eppool.tile([128, KH, 128], BF16, tag="t2sb")
                for k in range(KH):
                    tp2 = eppsum.tile([128, 128], BF16, tag="tr")
                    nc.tensor.transpose(
                        tp2[:], o1[:, k * 128:(k + 1) * 128], identb_sb[:])
                    nc.scalar.activation(t2sb[:, k, :], tp2[:], AF.Copy)
                h2ps = eppsum.tile([128, D_OUT], F32, tag="h2")
                for k in range(KH):
                    nc.tensor.matmul(h2ps[:], t2sb[:, k, :], w2_sb[:, k, :],
                                     start=(k == 0), stop=(k == KH - 1))
                h2sb = eppool.tile([128, D_L2], BF16, tag="h2sb")
                nc.vector.tensor_scalar(h2sb[:, :D_OUT], h2ps[:],
                                        dinvd_sb[:, t:t + 1], None, ALU.mult)
                nc.vector.memset(h2sb[:, D_OUT:], 0.0)
                nc.vector.tensor_copy(h2keep[:, t, :], h2sb[:, :D_OUT])
                nc.sync.dma_start(out=h2own_r[t], in_=h2sb[:])

            if 2 in phases:
                agg_phase(lambda g: xs_d.ap()[g * GR:(g + 1) * GR, :], D_IN,
                          BF16, D_IN, epi1, "m1", "s1", "ag1")

            # ---------------- AllGather h2 shards -------------------------
            if with_collective and 2 in phases:
                nc.gpsimd.collective_compute(
                    "AllGather", ALU.bypass,
                    replica_groups=[list(range(N_CORES))],
                    ins=[h2own.opt()], outs=[h2full.opt()])

            # ---------------- phase B: layer-2 agg + log_softmax ----------
            out_r = out_d.ap().rearrange("(t p) f -> t p f", p=128)

            def epi2(t, ps, eppool, eppsum):
                t0 = eppool.tile([128, D_OUT], F32, tag="t0")
                nc.vector.tensor_tensor(t0[:], ps, h2keep[:, t, :],
                                        ALU.add)
                nc.vector.tensor_scalar(t0[:], t0[:], dinvd_sb[:, t:t + 1],
                                        None, ALU.mult)
                nc.vector.tensor_tensor(t0[:], t0[:], b2_sb[:], ALU.add)
                nm = eppool.tile([128, 1], F32, tag="nm")
                nc.vector.tensor_reduce(nm[:], t0[:], mybir.AxisListType.X,
                                        ALU.max, negate=True)
                et = eppool.tile([128, D_OUT], F32, tag="et")
                se = eppool.tile([128, 1], F32, tag="se")
                nc.scalar.activation(et[:], t0[:], AF.Exp, bias=nm[:],
                                     accum_out=se[:])
                ls = eppool.tile([128, 1], F32, tag="ls")
                nc.scalar.activation(ls[:], se[:], AF.Ln)
                ot = eppool.tile([128, D_OUT], BF16, tag="ot")
                nc.vector.tensor_scalar(ot[:], t0[:], nm[:], ls[:],
                                        ALU.add, ALU.subtract)
                nc.sync.dma_start(out=out_r[t], in_=ot[:])

            if 3 in phases:
                agg_phase(lambda g: h2full[g * GR:(g + 1) * GR, :], D_L2,
                          BF16, D_OUT, epi2, "m2", "s2", "ag2")

    nc.compile()
    return nc


# --------------------------------------------------------------------------
# Entry point
# --------------------------------------------------------------------------

def kernel(x, edge_index, W1, b1, W2, b2):
    cfg = FULL_CFG
    in_maps, meta = preprocess(x, edge_index, W1, b1, W2, b2, cfg)
    nc = build_program(cfg, meta)
    res = run_bass_kernel_spmd(nc, in_maps, core_ids=list(range(N_CORES)))
    shards = [res.results[c]["out"] for c in range(N_CORES)]
    full = np.concatenate(shards, axis=0)
    return full[meta["perm"]].astype(np.float32)


# revision 40
# speedup vs baseline: 1.0654x; 1.0654x over previous
"""BasicGCN (2-layer GCN, 100K nodes / 3.2M edges) on 8 Trainium2 NeuronCores.

Strategy (node/dst sharding, graph-parallel), v3:
  - Pad nodes to NPAD = 100352 = 8 * 12544; core c owns dst rows
    [c*12544, (c+1)*12544). Nodes are snake-striped across cores by
    descending in-degree (host permutation, inverted on output) so
    per-(core,tile) edge counts equalize and the SPMD max-over-cores
    slot quotas stay tight.
  - Linearity refactor: segment_sum(dinv_s*x_s) @ W1 == the GCN conv, so
    the layer-1 gather table is just xs = dinv*x in bf16 (host-prepared).
    W1/relu/W2 are applied per dst tile in the phase-A epilogue (no
    dense replicated x@W1 phase at all).
  - Self-loops are NOT gather slots: the layer-1 self term (+= xs_own) is
    a per-tile DVE add in the epilogue; the layer-2 self term reuses the
    per-tile y values kept in SBUF (h2keep) from phase A.
  - Edge-granular slot quotas: per (dst-tile, src-group) section quota =
    max over cores of the edge count (no 128 roundup); sections are
    packed back-to-back inside each (tile-quad, group) run, runs padded
    to 128. A 128-slot block that straddles two sections gets TWO one-hot
    matmuls (one per dst tile) using two host-prepared dstloc columns
    (other-tile slots carry a sentinel -> zero S column).
  - Device per core:
      phase A (layer-1 agg): dma_gather xs rows (512B) in <=1024-row
        calls, one-hot S blocks on DVE, segment-sum via PE bf16 matmuls
        into per-tile f32 PSUM; epilogue: agg += xs_own,
        h1 = relu(dinv_d*(agg @ W1) + b1), y = dinv_d*(h1 @ W2) -> bf16
        shard table [12544, 128] (64 data + 64 zero pad) + SBUF copy
        (h2keep).
      AllGather shards -> h2full [NPAD, 128] bf16 (Shared DRAM).
      phase B (layer-2 agg): gather 256B bf16 rows, same S/matmul,
        epilogue adds h2keep self term + log_softmax -> out [12544, 64].
  - Host: concatenate 8 shards, trim to [100000, 64].

Gather tables are split into 4 row-groups of NPAD/4 = 25088 rows so the
int16 gather indices stay in range; each dma_gather call is capped at
QMAX=1024 indices (the q7 firmware breaks above ~1024) and spans a
quad's whole per-group run to keep calls full (SWDGE fixed cost is the
main Pool-engine expense).
"""

import numpy as np

import concourse.bacc as bacc
import concourse.bass as bass
import concourse.mybir as mybir
import concourse.tile as tile
from concourse.bass_utils import run_bass_kernel_spmd

F32 = mybir.dt.float32
BF16 = mybir.dt.bfloat16
I16 = mybir.dt.int16
NP_BF16 = mybir.dt.np(BF16)
AF = mybir.ActivationFunctionType
ALU = mybir.AluOpType

N_CORES = 8
PAD_DSTLOC = 1000.0  # sentinel dst-local for padding slots -> zero S column
QMAX = 1024  # max num_idxs per dma_gather call (HW limit is in (1024, 1280])
QT = 4       # dst tiles per gather bucket (quad)


def make_cfg(n_nodes=100000, d_in=256, d_hid=256, d_out=64, shard_tiles=98,
             n_groups=4):
    shard = shard_tiles * 128
    npad = N_CORES * shard
    assert npad % n_groups == 0
    gr = npad // n_groups
    assert gr <= 32768
    assert n_nodes <= npad
    return dict(N=n_nodes, NPAD=npad, SHARD=shard, NT=shard_tiles,
                NG=n_groups, GR=gr, D_IN=d_in, D_HID=d_hid, D_OUT=d_out)


FULL_CFG = make_cfg()


def _build_schedule(quota, nt, ng):
    """Gather-call schedule over (quad, group) runs, edge-granular quotas.

    Returns dict with:
      calls: list of (g, o, q) in stream order, q <= QMAX, q % 128 == 0
      emissions: per 128-block, list of (tile, dlcol) (1 or 2 entries)
      sec_off: [nt, ng] absolute slot offset of each section
      tile_of_slot: [slot_total] owning tile per slot (-1 for run pad)
      slot_total, n_dlcols
    """
    sec_off = np.zeros((nt, ng), np.int64)
    tile_parts = []
    emissions = []
    calls = []
    run_starts_acc = []
    n_dlcols = 0
    off = 0
    for qd in range(0, nt, QT):
        tiles = list(range(qd, min(qd + QT, nt)))
        for g in range(ng):
            run_start = off
            run_starts_acc.append(off)
            for t in tiles:
                q = int(quota[t, g])
                sec_off[t, g] = off
                tile_parts.append(np.full(q, t, np.int32))
                off += q
            pad = (-(off - run_start)) % 128
            if pad:
                tile_parts.append(np.full(pad, -1, np.int32))
                off += pad
            run_len = off - run_start
            if run_len == 0:
                continue
            # blocks + emissions for this run
            run_tiles = np.concatenate(tile_parts[-(len(tiles) + (1 if pad
                                                   else 0)):])
            for b in range(run_len // 128):
                blk_tiles = np.unique(run_tiles[b * 128:(b + 1) * 128])
                blk_tiles = [int(t) for t in blk_tiles if t >= 0]
                assert 1 <= len(blk_tiles) <= 2, blk_tiles
                ent = []
                for t in blk_tiles:
                    ent.append((t, n_dlcols))
                    n_dlcols += 1
                emissions.append(ent)
            # calls chunking the run
            import os
            nblk = run_len // 128
            nch = (run_len + QMAX - 1) // QMAX
            if os.environ.get("GCN_CHUNK", "full") == "even":
                base, rem = divmod(nblk, nch)
                o = run_start
                for i in range(nch):
                    q = (base + (1 if i < rem else 0)) * 128
                    calls.append((g, o, q))
                    o += q
            else:
                o = run_start
                left = run_len
                while left > 0:
                    q = min(QMAX, left)
                    calls.append((g, o, q))
                    o += q
                    left -= q
    tile_of_slot = np.concatenate(tile_parts)
    assert tile_of_slot.shape[0] == off
    return dict(calls=calls, emissions=emissions, sec_off=sec_off,
                tile_of_slot=tile_of_slot, slot_total=off,
                n_dlcols=n_dlcols, run_starts=run_starts_acc)


# --------------------------------------------------------------------------
# Host preprocessing
# --------------------------------------------------------------------------

def preprocess(x, edge_index, W1, b1, W2, b2, cfg):
    N, NPAD, SHARD, NT, NG, GR = (cfg["N"], cfg["NPAD"], cfg["SHARD"],
                                  cfg["NT"], cfg["NG"], cfg["GR"])
    D_IN, D_HID, D_OUT = cfg["D_IN"], cfg["D_HID"], cfg["D_OUT"]

    x = np.asarray(x, np.float32)
    edge_index = np.asarray(edge_index)
    src = edge_index[0].astype(np.int64)
    dst = edge_index[1].astype(np.int64)
    E = src.shape[0]

    # dst-balance permutation: snake-stripe nodes across cores by
    # in-degree so per-(core,tile) edge totals equalize across cores
    import os
    indeg = np.bincount(dst, minlength=N)
    iarr = np.arange(N)
    stripe, lane = iarr // N_CORES, iarr % N_CORES
    csel = np.where(stripe % 2 == 0, lane, N_CORES - 1 - lane)
    perm = np.empty(N, np.int64)
    perm[np.argsort(-indeg, kind="stable")] = csel * SHARD + stripe
    src = perm[src]
    dst = perm[dst]

    deg = np.bincount(dst, minlength=NPAD).astype(np.float32) + 1.0
    dinv_pad = 1.0 / np.sqrt(deg)  # [NPAD]; pad rows unused

    c_of = dst // SHARD
    t_of = (dst % SHARD) // 128
    d_of = (dst % 128).astype(np.float32)
    g_of = src // GR
    srcg = (src % GR).astype(np.int16)

    key = (c_of * NT + t_of) * NG + g_of
    order = np.argsort(key, kind="stable")
    counts = np.bincount(key, minlength=N_CORES * NT * NG)
    quota = counts.reshape(N_CORES, NT, NG).max(axis=0)

    sched = _build_schedule(quota, NT, NG)
    slot_total = sched["slot_total"]
    sec_off = sched["sec_off"]

    # slot position of each edge inside its core's stream
    csum = np.zeros(N_CORES * NT * NG + 1, np.int64)
    np.cumsum(counts, out=csum[1:])
    rank = np.arange(E, dtype=np.int64) - csum[key[order]]
    slot = sec_off[t_of[order], g_of[order]] + rank
    core = c_of[order]

    idx_arr = np.zeros((N_CORES, slot_total), np.int16)  # pad -> row 0
    dv_arr = np.full((N_CORES, slot_total), PAD_DSTLOC, np.float32)
    idx_arr[core, slot] = srcg[order]
    dv_arr[core, slot] = d_of[order]

    # idx wrapped [16, slot_total/16] (pos p -> [p%16, p//16]), replicated
    # to 128 partitions
    idx_sb = idx_arr.reshape(N_CORES, slot_total // 16, 16).transpose(
        0, 2, 1)
    idx_sb = np.tile(idx_sb, (1, 8, 1))

    # dstloc columns: one per emission; slots of other tiles -> sentinel
    n_dlcols = sched["n_dlcols"]
    tile_of_slot = sched["tile_of_slot"]
    dl_sb = np.full((N_CORES, 128, n_dlcols), PAD_DSTLOC, np.float32)
    blk = 0
    for ent in sched["emissions"]:
        sl = slice(blk * 128, (blk + 1) * 128)
        tslice = tile_of_slot[sl]
        for t, dlcol in ent:
            dl_sb[:, :, dlcol] = np.where(tslice[None, :] == t,
                                          dv_arr[:, sl], PAD_DSTLOC)
        blk += 1

    # gather table: xs = dinv * x, bf16, row-major (permuted node order)
    xs = np.zeros((NPAD, D_IN), NP_BF16)
    xs[perm] = (dinv_pad[perm][:, None] * x).astype(NP_BF16)

    ntile = NPAD // 128
    dinv_nodes = np.ascontiguousarray(
        dinv_pad.reshape(ntile, 128).T)  # [128, ntile]
    dinv_dst = np.stack([dinv_nodes[:, c * NT:(c + 1) * NT]
                         for c in range(N_CORES)])  # [8, 128, NT]

    iota = np.tile(np.arange(128), (128, 1)).astype(NP_BF16)
    ident_bf = np.eye(128, dtype=NP_BF16)
    b1bc = np.ascontiguousarray(np.broadcast_to(
        np.asarray(b1, np.float32), (128, D_HID))).astype(NP_BF16)
    b2bc = np.ascontiguousarray(
        np.broadcast_to(np.asarray(b2, np.float32), (128, D_OUT)))

    common = dict(xs=xs, W1=np.asarray(W1, NP_BF16),
                  W2=np.asarray(W2, NP_BF16), b1bc=b1bc, b2bc=b2bc,
                  iota=iota, ident_bf=ident_bf)
    in_maps = []
    for c in range(N_CORES):
        m = dict(common)
        m["xs_own"] = np.ascontiguousarray(
            xs[c * SHARD:(c + 1) * SHARD])
        m["dinv_dst"] = np.ascontiguousarray(dinv_dst[c])
        m["idx_sb"] = np.ascontiguousarray(idx_sb[c])
        m["dstloc"] = np.ascontiguousarray(dl_sb[c])
        in_maps.append(m)

    meta = dict(calls=sched["calls"], emissions=sched["emissions"],
                idxcols=slot_total // 16, n_dlcols=n_dlcols, perm=perm,
                run_starts=sched["run_starts"], slot_total=slot_total)
    return in_maps, meta


# --------------------------------------------------------------------------
# Device program
# --------------------------------------------------------------------------

def build_program(cfg, meta, with_collective=True, phases=(1, 2, 3)):
    NPAD, NT, NG, GR = cfg["NPAD"], cfg["NT"], cfg["NG"], cfg["GR"]
    D_IN, D_HID, D_OUT = cfg["D_IN"], cfg["D_HID"], cfg["D_OUT"]
    SHARD = cfg["SHARD"]
    idxcols, n_dlcols = meta["idxcols"], meta["n_dlcols"]
    calls, emissions = meta["calls"], meta["emissions"]
    # idx chunk boundaries (slot space): at run starts nearest k/NIC
    NIC = 8
    run_starts = meta["run_starts"]
    slot_total = meta["slot_total"]
    bounds = [0]
    for k in range(1, NIC):
        tgt = slot_total * k // NIC
        rs = min(run_starts, key=lambda r: abs(r - tgt))
        if rs > bounds[-1]:
            bounds.append(rs)
    bounds.append(slot_total)
    NIC = len(bounds) - 1
    KI = D_IN // 128   # k-chunks for W1 matmul
    KH = D_HID // 128  # k-chunks for W2 matmul
    CMAX = QMAX // 128
    D_L2 = 2 * D_OUT  # layer-2 table row: 64 bf16 data + 64 bf16 zeros

    # first/last emission index per tile (accum start/stop + epilogue)
    flat = [(bi, t) for bi, ent in enumerate(emissions) for (t, _) in ent]
    first_em = {}
    last_em = {}
    for i, (_, t) in enumerate(flat):
        first_em.setdefault(t, i)
        last_em[t] = i

    nc = bacc.Bacc("TRN2", target_bir_lowering=False, debug=False,
                   num_devices=N_CORES)

    xs_d = nc.dram_tensor("xs", [NPAD, D_IN], BF16, kind="ExternalInput")
    xso_d = nc.dram_tensor("xs_own", [SHARD, D_IN], BF16,
                           kind="ExternalInput")
    W1_d = nc.dram_tensor("W1", [D_IN, D_HID], BF16, kind="ExternalInput")
    W2_d = nc.dram_tensor("W2", [D_HID, D_OUT], BF16, kind="ExternalInput")
    b1_d = nc.dram_tensor("b1bc", [128, D_HID], BF16, kind="ExternalInput")
    b2_d = nc.dram_tensor("b2bc", [128, D_OUT], F32, kind="ExternalInput")
    iota_d = nc.dram_tensor("iota", [128, 128], BF16, kind="ExternalInput")
    identb_d = nc.dram_tensor("ident_bf", [128, 128], BF16,
                              kind="ExternalInput")
    dinvd_d = nc.dram_tensor("dinv_dst", [128, NT], F32, kind="ExternalInput")
    idx_d = nc.dram_tensor("idx_sb", [128, idxcols], I16, kind="ExternalInput")
    dl_d = nc.dram_tensor("dstloc", [128, n_dlcols], F32,
                          kind="ExternalInput")
    out_d = nc.dram_tensor("out", [SHARD, D_OUT], BF16,
                           kind="ExternalOutput")

    with tile.TileContext(nc) as tc:
        with (
            tc.tile_pool(name="const", bufs=1) as const,
            tc.tile_pool(name="dram", bufs=1, space="DRAM") as dram,
        ):
            h2own = dram.tile([SHARD, D_L2], BF16)
            h2full = dram.tile([NPAD, D_L2], BF16, addr_space="Shared")

            idx_tiles = []
            for k in range(NIC):
                c0, c1 = bounds[k] // 16, bounds[k + 1] // 16
                it = const.tile([128, c1 - c0], I16, name=f"idx{k}")
                nc.sync.dma_start(out=it[:], in_=idx_d.ap()[:, c0:c1])
                idx_tiles.append((bounds[k], bounds[k + 1], it))
                if k == 0:
                    dl_sb = const.tile([128, n_dlcols], F32)
                    nc.sync.dma_start(out=dl_sb[:], in_=dl_d.ap())

            def idx_slice(o, q):
                for b0, b1, it in idx_tiles:
                    if o >= b0 and o + q <= b1:
                        return it[:, (o - b0) // 16:(o + q - b0) // 16]
                raise AssertionError((o, q, bounds))

            w1_sb = const.tile([128, KI, D_HID], BF16)
            for k in range(KI):
                nc.sync.dma_start(out=w1_sb[:, k, :],
                                  in_=W1_d.ap()[k * 128:(k + 1) * 128, :])
            w2_sb = const.tile([128, KH, D_OUT], BF16)
            for k in range(KH):
                nc.sync.dma_start(out=w2_sb[:, k, :],
                                  in_=W2_d.ap()[k * 128:(k + 1) * 128, :])
            iota_sb = const.tile([128, 128], BF16)
            nc.sync.dma_start(out=iota_sb[:], in_=iota_d.ap())
            identb_sb = const.tile([128, 128], BF16)
            nc.sync.dma_start(out=identb_sb[:], in_=identb_d.ap())
            b1_sb = const.tile([128, D_HID], BF16)
            nc.sync.dma_start(out=b1_sb[:], in_=b1_d.ap())
            b2_sb = const.tile([128, D_OUT], F32)
            nc.sync.dma_start(out=b2_sb[:], in_=b2_d.ap())
            dinvd_sb = const.tile([128, NT], F32)
            nc.sync.dma_start(out=dinvd_sb[:], in_=dinvd_d.ap())
            h2keep = const.tile([128, NT, D_OUT], BF16)

            # ---------------- aggregation machinery ----------------------
            h2own_r = h2own.rearrange("(t p) f -> t p f", p=128)
            xso_r = xso_d.ap().rearrange("(t p) f -> t p f", p=128)

            def agg_phase(table, elem, edt, rhs_w, epilogue, mtag, stag,
                          ptag, prefetch=None, tail=None, tail_delay=32):
                """Gather + one-hot-S + matmul accumulation over the
                precomputed quad-spanning call schedule."""
                blk = 0
                em = 0
                psums = {}
                pending = []
                with (
                    tc.tile_pool(name=mtag, bufs=14) as mpool,
                    tc.tile_pool(name=stag, bufs=64) as spool,
                    tc.tile_pool(name=ptag, bufs=5, space="PSUM") as apsum,
                    tc.tile_pool(name=ptag + "ep", bufs=6) as eppool,
                    tc.tile_pool(name=ptag + "tr", bufs=1,
                                 space="PSUM") as trpsum,
                    tc.tile_pool(name=ptag + "ep2", bufs=1,
                                 space="PSUM") as eppsum,
                ):
                    for g, o, q, q_eff in calls:
                        ncols = q // 128
                        mt = mpool.tile([128, CMAX, elem], edt, tag="m")
                        nc.gpsimd.dma_gather(
                            mt[:, :ncols, :],
                            table(g),
                            idx_slice(o, q),
                            q_eff, q_eff, elem)
                        for j in range(ncols):
                            for t, dlcol in emissions[blk]:
                                if em == first_em[t]:
                                    psums[t] = apsum.tile(
                                        [128, rhs_w], F32, tag="agg",
                                        name="aggps")
                                    if prefetch is not None:
                                        prefetch(t, eppool)
                                st = spool.tile([128, 128], BF16, tag="s",
                                                name="stile")
                                nc.vector.tensor_scalar(
                                    st[:], iota_sb[:],
                                    dl_sb[:, dlcol:dlcol + 1],
                                    None, ALU.is_equal)
                                nc.tensor.matmul(
                                    psums[t][:], st[:], mt[:, j, :rhs_w],
                                    start=(em == first_em[t]),
                                    stop=(em == last_em[t]))
                                if em == last_em[t]:
                                    st8 = epilogue(t, psums.pop(t)[:],
                                                   eppool,
                                                   (trpsum, eppsum))
                                    if tail is not None:
                                        pending.append((blk, t, st8))
                                em += 1
                            while pending and \
                                    blk - pending[0][0] >= tail_delay:
                                _, t2, st2 = pending.pop(0)
                                tail(t2, st2, eppool)
                            blk += 1
                    while pending:
                        _, t2, st2 = pending.pop(0)
                        tail(t2, st2, eppool)

            # -------- phase A: layer-1 agg + full 2-layer epilogue --------
            xso_tiles = {}

            def pre1(t, eppool):
                xso = eppool.tile([128, D_IN], BF16, tag="xso")
                nc.sync.dma_start(out=xso[:], in_=xso_r[t])
                xso_tiles[t] = xso

            def epi1(t, ps, pools, unused=None):
                eppool, (trpsum, eppsum) = pools if isinstance(pools, tuple) \
                    else (pools, unused)
                # ps[d, f] = sum_e dinv_src * x_src  (agg of xs rows)
                xso = xso_tiles.pop(t)
                aggsb = eppool.tile([128, D_IN], BF16, tag="aggsb")
                nc.vector.tensor_tensor(aggsb[:], ps, xso[:], ALU.add)
                tsb = eppool.tile([128, KI, 128], BF16, tag="tsb")
                for k in range(KI):
                    tp = trpsum.tile([128, 128], BF16, tag="tr")
                    nc.tensor.transpose(
                        tp[:], aggsb[:, k * 128:(k + 1) * 128], identb_sb[:])
                    nc.vector.tensor_copy(tsb[:, k, :], tp[:])
                h1ps = eppsum.tile([128, D_HID], F32, tag="h1")
                for k in range(KI):
                    nc.tensor.matmul(h1ps[:], tsb[:, k, :], w1_sb[:, k, :],
                                     start=(k == 0), stop=(k == KI - 1))
                # h1 = relu(dinv_d * (agg @ W1) + b1), bf16
                o1 = eppool.tile([128, D_HID], BF16, tag="o1")
                nc.vector.tensor_scalar(o1[:], h1ps[:], dinvd_sb[:, t:t + 1],
                                        None, ALU.mult)
                nc.vector.tensor_tensor(o1[:], o1[:], b1_sb[:], ALU.add)
                nc.vector.tensor_scalar(o1[:], o1[:], 0.0, None, ALU.max)
                t2sb = eppool.tile([128, KH, 128], BF16, tag="t2sb")
                for k in range(KH):
                    tp2 = trpsum.tile([128, 128], BF16, tag="tr")
                    nc.tensor.transpose(
                        tp2[:], o1[:, k * 128:(k + 1) * 128], identb_sb[:])
                    nc.scalar.activation(t2sb[:, k, :], tp2[:], AF.Copy)
                h2ps = eppsum.tile([128, D_OUT], F32, tag="h2")
                for k in range(KH):
                    nc.tensor.matmul(h2ps[:], t2sb[:, k, :], w2_sb[:, k, :],
                                     start=(k == 0), stop=(k == KH - 1))
                h2sb = eppool.tile([128, D_L2], BF16, tag="h2sb")
                nc.vector.tensor_scalar(h2sb[:, :D_OUT], h2ps[:],
                                        dinvd_sb[:, t:t + 1], None, ALU.mult)
                nc.vector.memset(h2sb[:, D_OUT:], 0.0)
                nc.vector.tensor_copy(h2keep[:, t, :], h2sb[:, :D_OUT])
                nc.sync.dma_start(out=h2own_r[t], in_=h2sb[:])

            if 2 in phases:
                agg_phase(lambda g: xs_d.ap()[g * GR:(g + 1) * GR, :], D_IN,
                          BF16, D_IN, epi1, "m1", "s1", "ag1", prefetch=pre1)

            # ---------------- AllGather h2 shards -------------------------
            if with_collective and 2 in phases:
                nc.gpsimd.collective_compute(
                    "AllGather", ALU.bypass,
                    replica_groups=[list(range(N_CORES))],
                    ins=[h2own.opt()], outs=[h2full.opt()])

            # ---------------- phase B: layer-2 agg + log_softmax ----------
            out_r = out_d.ap().rearrange("(t p) f -> t p f", p=128)

            def epi2(t, ps, pools, unused=None):
                eppool = pools[0] if isinstance(pools, tuple) else pools
                t0 = eppool.tile([128, D_OUT], F32, tag="t0")
                nc.vector.tensor_tensor(t0[:], ps, h2keep[:, t, :],
                                        ALU.add)
                nc.vector.tensor_scalar(t0[:], t0[:], dinvd_sb[:, t:t + 1],
                                        None, ALU.mult)
                nc.vector.tensor_tensor(t0[:], t0[:], b2_sb[:], ALU.add)
                nm = eppool.tile([128, 1], F32, tag="nm")
                nc.vector.tensor_reduce(nm[:], t0[:], mybir.AxisListType.X,
                                        ALU.max, negate=True)
                et = eppool.tile([128, D_OUT], F32, tag="et")
                se = eppool.tile([128, 1], F32, tag="se")
                nc.scalar.activation(et[:], t0[:], AF.Exp, bias=nm[:],
                                     accum_out=se[:])
                return (t0, nm, se)

            def tail2(t, st8, eppool):
                t0, nm, se = st8
                ls = eppool.tile([128, 1], F32, tag="ls")
                nc.scalar.activation(ls[:], se[:], AF.Ln)
                ot = eppool.tile([128, D_OUT], BF16, tag="ot")
                nc.vector.tensor_scalar(ot[:], t0[:], nm[:], ls[:],
                                        ALU.add, ALU.subtract)
                nc.sync.dma_start(out=out_r[t], in_=ot[:])

            if 3 in phases:
                agg_phase(lambda g: h2full[g * GR:(g + 1) * GR, :], D_L2,
                          BF16, D_OUT, epi2, "m2", "s2", "ag2", tail=tail2)

    nc.compile()
    return nc


# --------------------------------------------------------------------------
# Entry point
# --------------------------------------------------------------------------

def kernel(x, edge_index, W1, b1, W2, b2):
    cfg = FULL_CFG
    in_maps, meta = preprocess(x, edge_index, W1, b1, W2, b2, cfg)
    nc = build_program(cfg, meta)
    res = run_bass_kernel_spmd(nc, in_maps, core_ids=list(range(N_CORES)))
    shards = [res.results[c]["out"] for c in range(N_CORES)]
    full = np.concatenate(shards, axis=0)
    return full[meta["perm"]].astype(np.float32)
